# revision 5
# baseline (speedup 1.0000x reference)
"""Trainium2 Bass kernel for fp8 (E4M3) quantized dense layer with bias.

Computes: out = fp8(x) @ fp8(W) + bias
  x: [32768, 1024] f32, W: [1024, 4096] f32, bias: [4096] f32 -> out [32768, 4096] f32

Sharding: data-parallel over tokens (32768/8 = 4096 tokens per core); W and bias
replicated. No collectives needed; per-core outputs concatenate along tokens.

Per-core pipeline (tokens processed in blocks of 128):
  1. DMA x block [128, 1024] f32 -> SBUF
  2. ACT cast f32 -> fp8e4 (TRN E4M3 == OCP E4M3FN for |v| <= 240; inputs ~N(0,1))
  3. Transpose via PE matmul-against-identity into [d, t] layout (exact), ACT
     copies PSUM f32 -> SBUF fp8 (exact: values are e4m3-representable)
  4. fp8 DoubleRow matmuls (K=256 per step) accumulate in PSUM f32
  5. DVE tensor_add applies bias (f32) while evicting PSUM -> SBUF
  6. DMA out block [128, 4096] f32 -> DRAM
"""

import os
import sys

for _p in ("/opt/trn_rl_repo", "/opt/pypackages"):
    if os.path.isdir(_p) and _p not in sys.path:
        sys.path.append(_p)

from contextlib import ExitStack

import numpy as np

import concourse.bass as bass
import concourse.mybir as mybir
import concourse.tile as tile
from concourse import bacc
from concourse.bass_utils import run_bass_kernel_spmd
from concourse.masks import make_identity

P = 128
D_MODEL = 1024
UNITS = 4096
TOKENS = 32768
N_CORES = 8
TPC = TOKENS // N_CORES  # tokens per core
N_FREE = 512  # psum bank free dim (f32)
F32 = mybir.dt.float32
FP8 = mybir.dt.float8e4

KS = D_MODEL // P  # 8 k-subtiles of 128
NU = UNITS // N_FREE  # 8 u-tiles of 512


def build_nc(tpc: int = TPC) -> bass.Bass:
    TB = tpc // P  # token blocks per core

    # Bacc (not plain Bass): its finalize runs generate_event_semaphores,
    # which splits multi-wait instructions — walrus allows only 1 wait/inst.
    nc = bacc.Bacc(
        "TRN2",
        target_bir_lowering=False,
        debug=False,
        enable_asserts=False,
        num_devices=N_CORES,
    )
    x_d = nc.declare_dram_parameter("x", [tpc, D_MODEL], F32, isOutput=False)
    w_d = nc.declare_dram_parameter("w", [D_MODEL, UNITS], F32, isOutput=False)
    b_d = nc.declare_dram_parameter("b", [P, UNITS], F32, isOutput=False)
    o_d = nc.declare_dram_parameter("out", [tpc, UNITS], F32, isOutput=True)

    # d = 128*s + p: partition p holds W rows {p, 128+p, ..., 896+p}
    w_view = w_d[:].rearrange("(s p) u -> p s u", p=P)

    with ExitStack() as ctx:
        tc = ctx.enter_context(tile.TileContext(nc))

        const = ctx.enter_context(tc.tile_pool(name="const", bufs=1))
        ident = const.tile([P, P], FP8)
        make_identity(nc, ident)
        bias_sb = const.tile([P, UNITS], F32)
        nc.sync.dma_start(bias_sb[:], b_d[:])

        w_fp8 = const.tile([P, KS, UNITS], FP8)
        wstage = ctx.enter_context(tc.tile_pool(name="wstage", bufs=2))
        for s in range(KS):
            st = wstage.tile([P, 1, UNITS], F32)
            nc.sync.dma_start(st[:], w_view[:, s : s + 1, :])
            nc.scalar.copy(w_fp8[:, s : s + 1, :], st[:])

        xin = ctx.enter_context(tc.tile_pool(name="xin", bufs=3))
        xqp = ctx.enter_context(tc.tile_pool(name="xq", bufs=3))
        xtp = ctx.enter_context(tc.tile_pool(name="xT", bufs=3))
        tps = ctx.enter_context(tc.tile_pool(name="tpsum", bufs=2, space="PSUM"))
        ops = ctx.enter_context(tc.tile_pool(name="opsum", bufs=4, space="PSUM"))
        outp = ctx.enter_context(tc.tile_pool(name="outp", bufs=3))

        for t in range(TB):
            xf = xin.tile([P, D_MODEL], F32)
            nc.sync.dma_start(xf[:], x_d[t * P : (t + 1) * P, :])
            xq = xqp.tile([P, D_MODEL], FP8)
            nc.scalar.copy(xq[:], xf[:])

            # xT[p, s, :] holds fp8 x.T for d = 128*s + p (matches w_view)
            xT = xtp.tile([P, KS, P], FP8)
            for h in range(KS // 4):
                pt = tps.tile([P, 4 * P], F32)
                for j in range(4):
                    s = 4 * h + j
                    nc.tensor.matmul(
                        pt[:, j * P : (j + 1) * P],
                        lhsT=xq[:, s * P : (s + 1) * P],
                        rhs=ident[:],
                        start=True,
                        stop=True,
                    )
                nc.scalar.copy(
                    xT[:, 4 * h : 4 * h + 4, :].rearrange("p a b -> p (a b)"),
                    pt[:],
                )

            ob = outp.tile([P, UNITS], F32)
            for u in range(NU):
                ps = ops.tile([P, N_FREE], F32)
                for k in range(KS // 2):
                    nc.tensor.matmul(
                        ps[:],
                        lhsT=xT[:, 2 * k : 2 * k + 2, :],
                        rhs=w_fp8[:, 2 * k : 2 * k + 2, u * N_FREE : (u + 1) * N_FREE],
                        start=(k == 0),
                        stop=(k == KS // 2 - 1),
                        perf_mode=mybir.MatmulPerfMode.DoubleRow,
                    )
                nc.vector.tensor_add(
                    ob[:, u * N_FREE : (u + 1) * N_FREE],
                    ps[:],
                    bias_sb[:, u * N_FREE : (u + 1) * N_FREE],
                )
            nc.sync.dma_start(o_d[t * P : (t + 1) * P, :], ob[:])

    nc.finalize()
    return nc


_NC_CACHE: dict = {}


def _get_nc(tpc: int = TPC) -> bass.Bass:
    if tpc not in _NC_CACHE:
        _NC_CACHE[tpc] = build_nc(tpc)
    return _NC_CACHE[tpc]


def run(x, w, bias, trace: bool = False, **kwargs):
    """Shard, execute on 8 cores, gather. Returns (out, BassKernelResults)."""
    x = np.ascontiguousarray(np.asarray(x, dtype=np.float32))
    w = np.ascontiguousarray(np.asarray(w, dtype=np.float32))
    bias = np.asarray(bias, dtype=np.float32).reshape(UNITS)
    b = np.ascontiguousarray(np.broadcast_to(bias[None, :], (P, UNITS)))

    nc = _get_nc(TPC)
    in_maps = [
        {"x": x[c * TPC : (c + 1) * TPC], "w": w, "b": b} for c in range(N_CORES)
    ]
    res = run_bass_kernel_spmd(
        nc, in_maps, list(range(N_CORES)), trace=trace, **kwargs
    )
    out = np.concatenate([r["out"] for r in res.results], axis=0)
    return out, res


def kernel(x, kernel, bias):  # noqa: A002 - harness-specified parameter names
    out, _ = run(x, kernel, bias)
    return out
